# revision 51
# baseline (speedup 1.0000x reference)
"""Cross-attention Trainium2 Bass kernel.

Reference computation (per batch b):
    q = relu(scale_q * (Wq @ qf) + bias_q)          [C, Nq]
    k = relu(scale_k * (Wk @ kf) + bias_k)          [C, Nk]
    v = relu(scale_v * (Wv @ kf) + bias_v)          [C, Nk]
    sim  = q.T @ k / sqrt(C)                        [Nq, Nk]
    attn = softmax(sim, axis=-1)
    ctx  = v @ attn.T                               [C, Nq]

Sharding: 8 cores = 4 batches x 2 query halves (Nq 4096 -> 2048 per core).
Each core gets the full K/V for its batch (recomputed, cheap) and half the
query positions; output halves are concatenated on the host.

Device-side design (per core):
  - BN scale folded into the weights on the host; weights fed pre-transposed
    and host-packed to the SBUF partition layout so every input DMA moves
    128 long contiguous rows (few, large descriptors).
  - all matmul operands in bfloat16 (same PE rate as f32r at 1 cycle/row,
    half the DMA/SBUF traffic and LDWEIGHTS time); PSUM accumulation f32.
    Measured rel err vs the f32 reference: 3.7e-4 (gate is 2e-2).
  - sim is computed transposed (k on partitions, q on free dim) so the
    exp(sim) tiles feed the ctx matmul as the moving operand directly -- no
    attention transpose is ever needed.
  - softmax uses a constant shift instead of a row max: exp(sim/sqrt(C) - 4)
    (sim/sqrt(C) is bounded by ~|q||k|/16 << 88, so no overflow is possible).
  - softmax row sums: the exp tiles are pairwise-summed on DVE (bf16 merge
    tree) and reduced across partitions by a single [P,1]x[P,QC] matmul per
    chunk -- instead of a 2-row PE matmul per key tile, which wasted a full
    512-row PE pass for 2/128 output partitions (25% of sim+ctx time).
  - the reciprocal -> partition-broadcast (K=1 matmul) -> copy chain runs at
    the START of the chunk's ctx phase (sums are ready as soon as the exps
    are), so the per-chunk PE stall waiting on DVE's slow reciprocal is gone.
  - ctx is accumulated unnormalized; normalization multiplies by 1/sums
    broadcast across partitions; per-ct output DMAs overlap the muls.
  - a burst of N_WARM=26 throwaway matmuls issued first locks the tensor
    engine's fast clock state (steady 221ns/512-row matmul vs a permanent
    ~263ns plateau without it, +29us end-to-end; 12 matmuls do NOT trigger
    it). The burst overlaps the input-DMA window.
"""

import sys

for _p in ("/opt/trn_rl_repo", "/root/.axon_site/_ro/trn_rl_repo"):
    if _p not in sys.path:
        sys.path.insert(0, _p)

import numpy as np

import concourse.bacc as bacc
import concourse.mybir as mybir
import concourse.tile as tile
from concourse.bass_utils import run_bass_kernel_spmd

F32 = mybir.dt.float32
F32R = mybir.dt.float32r
BF16 = mybir.dt.bfloat16
AF = mybir.ActivationFunctionType
ADD = mybir.AluOpType.add

B, C, H, W = 4, 256, 64, 64
NK = H * W          # 4096 key positions per batch
NQ = NK // 2        # 2048 query positions per core
P = 128
CO = C // P         # 2 contraction subtiles
QC = 512            # query chunk (matmul moving free dim)
NQC = NQ // QC      # 4 query chunks per core
KT = NK // P        # 32 key tiles
NP = KT // 2        # 16 key-tile pairs
EXP_SHIFT = -4.0    # exp(sim/sqrt(C) + EXP_SHIFT); sim/sqrt(C) observed in [0.5, 7.5]
SCALE = 1.0 / np.sqrt(C)
BC_KP = 10          # ctx pair index at which the 1/sums broadcast is emitted
QP = 2              # qf DMA pieces (host-packed rows)
KP_ = 4             # kf DMA pieces
N_WARM = 26         # PE warm-up matmuls (~11us at ramping clock)


def _build_program():
    nc = bacc.Bacc("TRN2", target_bir_lowering=False, debug=False)

    # feature/weight inputs are host-packed to the SBUF partition layout so
    # every DMA moves 128 long contiguous rows (few, large descriptors)
    qf = nc.dram_tensor("qf", [P, QP, CO, NQ // QP], BF16,
                        kind="ExternalInput").ap()
    kf = nc.dram_tensor("kf", [P, KP_, CO, NK // KP_], BF16,
                        kind="ExternalInput").ap()
    wqT = nc.dram_tensor("wqT", [P, CO, C], BF16, kind="ExternalInput").ap()
    wkT = nc.dram_tensor("wkT", [P, CO, C], BF16, kind="ExternalInput").ap()
    wvT = nc.dram_tensor("wvT", [P, CO, C], BF16, kind="ExternalInput").ap()
    bq = nc.dram_tensor("bq", [P, CO], F32, kind="ExternalInput").ap()
    bk = nc.dram_tensor("bk", [P, CO], F32, kind="ExternalInput").ap()
    bvb = nc.dram_tensor("bvb", [P, C], F32, kind="ExternalInput").ap()
    ones = nc.dram_tensor("ones", [1, P], F32R, kind="ExternalInput").ap()
    # output likewise packed [ci, qc, ct, n'], in bf16 (halves the final
    # store; quantization adds ~1e-3 norm err vs a 2e-2 gate); host unpacks
    out = nc.dram_tensor("out", [P, NQC, CO, QC], BF16,
                         kind="ExternalOutput").ap()

    with tile.TileContext(nc) as tc:
        with (
            nc.allow_low_precision(reason="fp32r/bf16 matmul operands"),
            tc.tile_pool(name="consts", bufs=1) as consts,
            tc.tile_pool(name="persist", bufs=1) as persist,
        ):
            # ---- constants (issue order matters: the first projection only
            # needs wqT + bq + the first qf chunk, so those go first and the
            # remaining weights ride behind the qf/kf streams) ----
            wqT_sb = consts.tile([P, CO, C], BF16, name="wqT_sb")
            nc.gpsimd.dma_start(wqT_sb[:], wqT[:])
            bq_sb = consts.tile([P, CO], F32, name="bq_sb")
            wkT_sb = consts.tile([P, CO, C], BF16, name="wkT_sb")
            wvT_sb = consts.tile([P, CO, C], BF16, name="wvT_sb")
            bk_sb = consts.tile([P, CO], F32, name="bk_sb")
            bvb_sb = consts.tile([P, C], F32, name="bvb_sb")
            ones_sb = consts.tile([1, P], F32R, name="ones_sb")
            # ---- PE warm-up, issued FIRST so PE starts the moment its
            # sequencer comes up. A burst of >=~26 junk matmuls at kernel
            # start locks the tensor engine's fast clock state (steady
            # 221ns/512-row matmul vs a permanent 263ns plateau without it:
            # +29us whole-kernel). 12 matmuls are NOT enough to trigger it.
            # Results are never read; the PSUM bank is reclaimed on pool
            # close. memset only supports f32 set-values, so the f32r tile
            # is filled via a converting copy.
            junk_f = consts.tile([P, QC], F32, name="junk_f")
            nc.vector.memset(junk_f[:], 1.0)
            junk_sb = consts.tile([P, QC], F32R, name="junk_sb")
            nc.vector.tensor_copy(out=junk_sb[:], in_=junk_f[:])
            with tc.tile_pool(name="warm_ps", bufs=1, space="PSUM") as warm_pool:
                warm_ps = warm_pool.tile([1, QC], F32, name="warm_ps")
                for i in range(N_WARM):
                    nc.tensor.matmul(
                        warm_ps[:], junk_sb[:, :1], junk_sb[:],
                        start=True, stop=True, skip_group_check=True,
                    )

            b0_sb = consts.tile([P, 1], F32, name="b0_sb")
            nc.vector.memset(b0_sb[:], EXP_SHIFT)
            onec_sb = consts.tile([P, 1], BF16, name="onec_sb")
            nc.vector.tensor_copy(out=onec_sb[:], in_=junk_f[:, :1])
            # dummy activation: pulls the ~1.3us LoadActFuncSet into the
            # initial DMA-wait window instead of blocking the first relu
            warm_sb = consts.tile([P, 1], F32, name="warm_sb")
            nc.scalar.activation(warm_sb[:], b0_sb[:], AF.Relu)

            # ---- persistent activations ----
            q_sb = persist.tile([P, CO, NQ], BF16, name="q_sb")
            k_sb = persist.tile([P, CO, NK], BF16, name="k_sb")
            vT_sb = persist.tile([P, KT, C], BF16, name="vT_sb")

            # ---- projections (staging pool scoped so its SBUF is reused) ----
            with (
                tc.tile_pool(name="staging", bufs=1) as staging,
                tc.tile_pool(name="proj_ps", bufs=1, space="PSUM") as proj_ps,
            ):
                # Input DMA plan. Each dma_start costs ~650ns of serial SP
                # dispatch, so: few DMAs, the first-needed bytes first
                # (wqT+bq+qf chunk 1), and strictly need-before order.
                qf_sb = staging.tile([P, CO, NQ], BF16, name="qf_sb")
                kf_sb = staging.tile([P, CO, NK], BF16, name="kf_sb")
                nqp, nkp = NQ // QP, NK // KP_
                nc.gpsimd.dma_start(bq_sb[:], bq[:])
                for p in range(QP):
                    nc.sync.dma_start(
                        qf_sb[:, :, p * nqp:(p + 1) * nqp], qf[:, p])
                nc.sync.dma_start(kf_sb[:, :, :nkp], kf[:, 0])
                nc.gpsimd.dma_start(wkT_sb[:], wkT[:])
                # wvT ahead of the tiny bias transfers: its late arrival was
                # the suspected cause of a recurring stall before the first
                # v-proj matmul (each dma_start is ~650ns of serial dispatch)
                nc.gpsimd.dma_start(wvT_sb[:], wvT[:])
                nc.gpsimd.dma_start(bk_sb[:], bk[:])
                nc.gpsimd.dma_start(bvb_sb[:], bvb[:])
                nc.gpsimd.dma_start(ones_sb[:], ones[:])
                for p in range(1, KP_):
                    nc.sync.dma_start(
                        kf_sb[:, :, p * nkp:(p + 1) * nkp], kf[:, p])

                def proj_iter(j, w_sb, bias_sb, dst, src_sb, oo1_act=False):
                    # one [*, QC] chunk of a q/k projection; relu+bias for
                    # oo=0 runs on ACT, oo=1 on DVE (or ACT for a couple of
                    # chunks, to even out the two engines' proj-phase load)
                    for oo in range(CO):
                        ps = proj_ps.tile([P, QC], F32, tag="pj", bufs=2,
                                          name=f"ps_{j}_{oo}")
                        for co in range(CO):
                            nc.tensor.matmul(
                                ps[:],
                                w_sb[:, co, oo * P:(oo + 1) * P],
                                src_sb[:, co, j * QC:(j + 1) * QC],
                                start=(co == 0), stop=(co == CO - 1),
                            )
                        if oo == 0 or oo1_act:
                            nc.scalar.activation(
                                dst[:, oo, j * QC:(j + 1) * QC], ps[:], AF.Relu,
                                bias=bias_sb[:, oo:oo + 1],
                            )
                        else:
                            nc.vector.tensor_scalar(
                                dst[:, oo, j * QC:(j + 1) * QC], ps[:],
                                bias_sb[:, oo:oo + 1], 0.0,
                                mybir.AluOpType.add, mybir.AluOpType.max,
                            )

                def vt_pair(kp):
                    # vT = relu(kf.T @ Wv'.T + bias_v): [n, o], n on partitions.
                    # bias_v varies along the free dim here, so it is added
                    # from a host-broadcast tile on DVE, then relu on ACT
                    # (writing bf16) -- no PE bias matmul needed.
                    psv = proj_ps.tile([P, 2, C], F32, tag="pv", bufs=3,
                                       name=f"psv_{kp}")
                    for half in range(2):
                        kt = 2 * kp + half
                        for co in range(CO):
                            nc.tensor.matmul(
                                psv[:, half, :],
                                kf_sb[:, co, kt * P:(kt + 1) * P],
                                wvT_sb[:, co, :],
                                start=(co == 0), stop=(co == CO - 1),
                            )
                    vtmp = staging.tile([P, 2, C], F32, tag="vtmp", bufs=3,
                                        name=f"vtmp_{kp}")
                    nc.vector.tensor_tensor(
                        vtmp[:], psv[:],
                        bvb_sb[:, None, :].to_broadcast((P, 2, C)),
                        ADD,
                    )
                    nc.scalar.activation(
                        vT_sb[:, 2 * kp:2 * kp + 2, :], vtmp[:], AF.Relu)

                # q = relu(Wq' @ qf + bq): [o, n] with o on partitions.
                # j-major so each arriving qf/kf chunk is fully consumed at
                # once; the vT pairs for chunk j of kf ride along with proj-k
                # so PE work fills the relu latency.
                for j in range(NQ // QC):
                    proj_iter(j, wqT_sb, bq_sb, q_sb, qf_sb)
                for j in range(NK // QC):
                    proj_iter(j, wkT_sb, bk_sb, k_sb, kf_sb,
                              oo1_act=j in (3, 6))
                    vt_pair(2 * j)
                    vt_pair(2 * j + 1)

            # ---- attention ----
            with (
                tc.tile_pool(name="expp", bufs=1) as expp,
                tc.tile_pool(name="outp", bufs=1) as outp,
                tc.tile_pool(name="attn_ps", bufs=1, space="PSUM") as attn_ps,
            ):
                # Software pipeline: step s emits sim+exp for chunk s
                # interleaved (at k-pair granularity) with the ctx matmuls
                # consuming chunk s-1's exp tiles. PE's ctx work fills the
                # ACT-exp latency that otherwise stalls the sim phase, and
                # the per-chunk phase boundaries disappear.
                exp_pairs = {}     # qc -> list of pair tiles
                sum_tiles = {}     # qc -> [P, QC] bf16 total of all 32 exps
                carry = {}         # qc -> binary-counter merge tree state

                def emit_sim_pair(qc, kp):
                    qs = slice(qc * QC, (qc + 1) * QC)
                    ps = attn_ps.tile([P, 2, QC], F32, tag="sim", bufs=2,
                                      name=f"pss_{qc}_{kp}")
                    for half in range(2):
                        kt = 2 * kp + half
                        for co in range(CO):
                            nc.tensor.matmul(
                                ps[:, half, :],
                                k_sb[:, co, kt * P:(kt + 1) * P],
                                q_sb[:, co, qs],
                                start=(co == 0), stop=(co == CO - 1),
                            )
                    et = expp.tile([P, 2, QC], BF16, tag="expT", bufs=24,
                                   name=f"expT_{qc}_{kp}")
                    nc.scalar.activation(et[:], ps[:], AF.Exp,
                                         bias=b0_sb[:], scale=float(SCALE))
                    exp_pairs.setdefault(qc, []).append(et)
                    # row-sum accumulation on DVE (bf16, 4x element mode):
                    # halves-add, then binary-counter merge. After the last
                    # pair, carry[4] holds the chunk's full exp sum.
                    h = expp.tile([P, QC], BF16, tag="hsum", bufs=3,
                                  name=f"h_{qc}_{kp}")
                    nc.vector.tensor_tensor(h[:], et[:, 0, :], et[:, 1, :], ADD)
                    node, lvl, c = h, 0, carry[qc]
                    while c[lvl] is not None:
                        prev, c[lvl] = c[lvl], None
                        nxt = expp.tile([P, QC], BF16, tag=f"lv{lvl + 1}",
                                        bufs=2, name=f"s_{qc}_{kp}_{lvl + 1}")
                        nc.vector.tensor_tensor(nxt[:], prev[:], node[:], ADD)
                        node, lvl = nxt, lvl + 1
                    c[lvl] = node
                    if kp == NP - 1:
                        sum_tiles[qc] = c[4]

                def emit_ctx_pair(qc, kp, ctx_ps):
                    for half in range(2):
                        kt = 2 * kp + half
                        e = exp_pairs[qc][kp][:, half, :]
                        for ct in range(CO):
                            nc.tensor.matmul(
                                ctx_ps[ct][:],
                                vT_sb[:, kt, ct * P:(ct + 1) * P],
                                e,
                                start=(kt == 0), stop=(kt == KT - 1),
                                skip_group_check=True,
                            )

                def emit_norm_sums(qcp):
                    # sums -> reciprocal, early: S was finished by DVE at the
                    # end of the previous step, so the slow reciprocal
                    # overlaps the ctx matmuls instead of stalling the tail.
                    S = sum_tiles.pop(qcp)
                    sums_ps = attn_ps.tile([1, QC], F32, tag="sums", bufs=2,
                                           name=f"psS_{qcp}")
                    nc.tensor.matmul(sums_ps[:], onec_sb[:], S[:],
                                     start=True, stop=True)
                    recip = outp.tile([1, QC], F32R, tag="recip",
                                      bufs=2, name=f"recip_{qcp}")
                    nc.vector.reciprocal(recip[:], sums_ps[:])
                    return recip

                def emit_norm_bcast(qcp, recip):
                    # broadcast 1/sums to all partitions (K=1 matmul) once
                    # the reciprocal has finished
                    bc_ps = attn_ps.tile([P, QC], F32, tag="sim",
                                         bufs=2, name=f"psb_{qcp}")
                    nc.tensor.matmul(bc_ps[:], ones_sb[:], recip[:],
                                     start=True, stop=True)
                    bc_sb = outp.tile([P, QC], F32, tag="bc",
                                      bufs=2, name=f"bc_{qcp}")
                    nc.vector.tensor_copy(out=bc_sb[:], in_=bc_ps[:])
                    return bc_sb

                def emit_last_chunk(qcp, ctx_ps):
                    # Final chunk, ct-major: ct0's full 32-tile accumulation,
                    # normalize and store drain underneath ct1's matmuls, so
                    # the end-of-kernel chain is only ct1's mul + DMA.
                    # sums is emitted after ct0's second pair (not first):
                    # chunk 3's DVE sum tree finishes ~1.4us after its last
                    # sim matmul, so a leading sums matmul stalls PE at the
                    # step boundary -- the generic path's kp==1 placement is
                    # measured stall-free.
                    recip = None
                    ot = outp.tile([P, CO, QC], BF16, tag="out", bufs=2,
                                   name=f"out_{qcp}")
                    bc_sb = None
                    for ct in range(CO):
                        for kp in range(NP):
                            if ct == 0 and kp == 2:
                                recip = emit_norm_sums(qcp)
                            for half in range(2):
                                kt = 2 * kp + half
                                e = exp_pairs[qcp][kp][:, half, :]
                                nc.tensor.matmul(
                                    ctx_ps[ct][:],
                                    vT_sb[:, kt, ct * P:(ct + 1) * P],
                                    e,
                                    start=(kt == 0), stop=(kt == KT - 1),
                                    skip_group_check=True,
                                )
                        if ct == 0:
                            # broadcast between the ct passes: PE arrives
                            # ~7us into the step, well after the reciprocal
                            # retires, so no stall; ct0's mul+store still
                            # drain under ct1's matmuls
                            bc_sb = emit_norm_bcast(qcp, recip)
                            nc.vector.tensor_mul(ot[:, ct, :], ctx_ps[ct][:],
                                                 bc_sb[:])
                            nc.sync.dma_start(out[:, qcp, ct], ot[:, ct, :])
                        else:
                            # very last store, in column halves: the first
                            # half's DMA rides under the second half's mul
                            hq = QC // 2
                            for hh in range(2):
                                cs = slice(hh * hq, (hh + 1) * hq)
                                nc.vector.tensor_mul(
                                    ot[:, ct, cs], ctx_ps[ct][:, cs],
                                    bc_sb[:, cs])
                                nc.sync.dma_start(out[:, qcp, ct, cs],
                                                  ot[:, ct, cs])
                    exp_pairs.pop(qcp)

                ctx_live = None  # (qc, ctx_ps, recip, [bc_sb]) being accumulated
                for s in range(NQC + 1):
                    if s > 0:
                        qcp = s - 1
                        ctx_ps = [
                            attn_ps.tile([P, QC], F32, tag="ctx", bufs=2,
                                         name=f"psc_{qcp}_{ct}")
                            for ct in range(CO)
                        ]
                        ctx_live = (qcp, ctx_ps, None, [None])
                    if s < NQC:
                        carry[s] = [None] * 5
                    if s == NQC:
                        emit_last_chunk(qcp, ctx_ps)
                        ctx_live = None
                        break
                    for kp in range(NP):
                        if s < NQC:
                            emit_sim_pair(s, kp)
                        if ctx_live is not None:
                            qcp, ctx_ps, recip, bc_box = ctx_live
                            emit_ctx_pair(qcp, kp, ctx_ps)
                            if kp == 1:
                                recip = emit_norm_sums(qcp)
                                ctx_live = (qcp, ctx_ps, recip, bc_box)
                            elif kp == BC_KP:
                                bc_box[0] = emit_norm_bcast(qcp, recip)
                    if ctx_live is not None:
                        qcp, ctx_ps, recip, bc_box = ctx_live
                        ot = outp.tile([P, CO, QC], BF16, tag="out", bufs=2,
                                       name=f"out_{qcp}")
                        # per-ct DMA so the ct=0 store overlaps the ct=1 mul
                        # (shortens the end-of-kernel chain)
                        for ct in range(CO):
                            nc.vector.tensor_mul(ot[:, ct, :], ctx_ps[ct][:],
                                                 bc_box[0][:])
                            nc.sync.dma_start(out[:, qcp, ct], ot[:, ct, :])
                        exp_pairs.pop(qcp)
                        ctx_live = None

    nc.compile()
    return nc


_PROGRAM = None


def _get_program():
    global _PROGRAM
    if _PROGRAM is None:
        _PROGRAM = _build_program()
    return _PROGRAM


def _prepare_in_maps(
    query_feats, key_feats, Wq, Wk, Wv,
    scale_q, bias_q, scale_k, bias_k, scale_v, bias_v,
):
    import ml_dtypes

    bf16 = ml_dtypes.bfloat16
    f32 = np.float32
    qf_all = np.asarray(query_feats, f32).reshape(B, C, NK)
    kf_all = np.asarray(key_feats, f32).reshape(B, C, NK)

    def pack_feat(x, pieces):
        # [C, N] -> [ci, piece, co, n'] so each DMA piece is 128 long rows
        n = x.shape[1]
        return np.ascontiguousarray(
            x.reshape(CO, P, pieces, n // pieces).transpose(1, 2, 0, 3)
        ).astype(bf16)

    def pack_w(wT):
        # [C, C] -> [ci, co, o]
        return np.ascontiguousarray(
            wT.reshape(CO, P, C).transpose(1, 0, 2)).astype(bf16)

    wqT = pack_w((np.asarray(scale_q, f32)[:, None] * np.asarray(Wq, f32)).T)
    wkT = pack_w((np.asarray(scale_k, f32)[:, None] * np.asarray(Wk, f32)).T)
    wvT = pack_w((np.asarray(scale_v, f32)[:, None] * np.asarray(Wv, f32)).T)
    bq2 = np.ascontiguousarray(np.asarray(bias_q, f32).reshape(CO, P).T)
    bk2 = np.ascontiguousarray(np.asarray(bias_k, f32).reshape(CO, P).T)
    bvb = np.broadcast_to(np.asarray(bias_v, f32)[None, :], (P, C)).copy()
    ones = np.ones((1, P), f32)

    shared = dict(wqT=wqT, wkT=wkT, wvT=wvT, bq=bq2, bk=bk2,
                  bvb=bvb, ones=ones)
    in_maps = []
    for core in range(8):
        b, h = divmod(core, 2)
        in_maps.append(dict(
            qf=pack_feat(qf_all[b][:, h * NQ:(h + 1) * NQ], QP),
            kf=pack_feat(kf_all[b], KP_),
            **shared,
        ))
    return in_maps


def run(inputs: dict, trace: bool = False):
    """Compile (cached) + run on 8 cores. Returns (output, BassKernelResults)."""
    nc = _get_program()
    in_maps = _prepare_in_maps(**inputs)
    res = run_bass_kernel_spmd(nc, in_maps, core_ids=list(range(8)), trace=trace)
    full = np.empty((B, C, NK), np.float32)
    for core in range(8):
        b, h = divmod(core, 2)
        # device wrote bf16 [ci, qc, ct, n']; un-permute to f32 [C, NQ]
        o = np.asarray(res.results[core]["out"],
                       np.float32).reshape(P, NQC, CO, QC)
        full[b][:, h * NQ:(h + 1) * NQ] = (
            o.transpose(2, 0, 1, 3).reshape(C, NQ))
    return full.reshape(B, C, H, W), res


def kernel(**inputs) -> np.ndarray:
    return run(inputs)[0]


# revision 52
# speedup vs baseline: 1.0007x; 1.0007x over previous
"""Cross-attention Trainium2 Bass kernel.

Reference computation (per batch b):
    q = relu(scale_q * (Wq @ qf) + bias_q)          [C, Nq]
    k = relu(scale_k * (Wk @ kf) + bias_k)          [C, Nk]
    v = relu(scale_v * (Wv @ kf) + bias_v)          [C, Nk]
    sim  = q.T @ k / sqrt(C)                        [Nq, Nk]
    attn = softmax(sim, axis=-1)
    ctx  = v @ attn.T                               [C, Nq]

Sharding: 8 cores = 4 batches x 2 query halves (Nq 4096 -> 2048 per core).
Each core gets the full K/V for its batch (recomputed, cheap) and half the
query positions; output halves are concatenated on the host.

Device-side design (per core):
  - BN scale folded into the weights on the host; weights fed pre-transposed
    and host-packed to the SBUF partition layout so every input DMA moves
    128 long contiguous rows (few, large descriptors).
  - all matmul operands in bfloat16 (same PE rate as f32r at 1 cycle/row,
    half the DMA/SBUF traffic and LDWEIGHTS time); PSUM accumulation f32.
    Measured rel err vs the f32 reference: 3.7e-4 (gate is 2e-2).
  - sim is computed transposed (k on partitions, q on free dim) so the
    exp(sim) tiles feed the ctx matmul as the moving operand directly -- no
    attention transpose is ever needed.
  - softmax uses a constant shift instead of a row max: exp(sim/sqrt(C) - 4)
    (sim/sqrt(C) is bounded by ~|q||k|/16 << 88, so no overflow is possible).
  - softmax row sums: the exp tiles are pairwise-summed on DVE (bf16 merge
    tree) and reduced across partitions by a single [P,1]x[P,QC] matmul per
    chunk -- instead of a 2-row PE matmul per key tile, which wasted a full
    512-row PE pass for 2/128 output partitions (25% of sim+ctx time).
  - the reciprocal -> partition-broadcast (K=1 matmul) -> copy chain runs at
    the START of the chunk's ctx phase (sums are ready as soon as the exps
    are), so the per-chunk PE stall waiting on DVE's slow reciprocal is gone.
  - ctx is accumulated unnormalized; normalization multiplies by 1/sums
    broadcast across partitions; per-ct output DMAs overlap the muls.
  - a burst of N_WARM=26 throwaway matmuls issued first locks the tensor
    engine's fast clock state (steady 221ns/512-row matmul vs a permanent
    ~263ns plateau without it, +29us end-to-end; 12 matmuls do NOT trigger
    it). The burst overlaps the input-DMA window.
"""

import sys

for _p in ("/opt/trn_rl_repo", "/root/.axon_site/_ro/trn_rl_repo"):
    if _p not in sys.path:
        sys.path.insert(0, _p)

import numpy as np

import concourse.bacc as bacc
import concourse.mybir as mybir
import concourse.tile as tile
from concourse.bass_utils import run_bass_kernel_spmd

F32 = mybir.dt.float32
F32R = mybir.dt.float32r
BF16 = mybir.dt.bfloat16
AF = mybir.ActivationFunctionType
ADD = mybir.AluOpType.add

B, C, H, W = 4, 256, 64, 64
NK = H * W          # 4096 key positions per batch
NQ = NK // 2        # 2048 query positions per core
P = 128
CO = C // P         # 2 contraction subtiles
QC = 512            # query chunk (matmul moving free dim)
NQC = NQ // QC      # 4 query chunks per core
KT = NK // P        # 32 key tiles
NP = KT // 2        # 16 key-tile pairs
EXP_SHIFT = -4.0    # exp(sim/sqrt(C) + EXP_SHIFT); sim/sqrt(C) observed in [0.5, 7.5]
SCALE = 1.0 / np.sqrt(C)
BC_KP = 10          # ctx pair index at which the 1/sums broadcast is emitted
QP = 2              # qf DMA pieces (host-packed rows)
KP_ = 4             # kf DMA pieces
N_WARM = 26         # PE warm-up matmuls (~11us at ramping clock)


def _build_program():
    nc = bacc.Bacc("TRN2", target_bir_lowering=False, debug=False)

    # feature/weight inputs are host-packed to the SBUF partition layout so
    # every DMA moves 128 long contiguous rows (few, large descriptors)
    qf = nc.dram_tensor("qf", [P, QP, CO, NQ // QP], BF16,
                        kind="ExternalInput").ap()
    kf = nc.dram_tensor("kf", [P, KP_, CO, NK // KP_], BF16,
                        kind="ExternalInput").ap()
    wqT = nc.dram_tensor("wqT", [P, CO, C], BF16, kind="ExternalInput").ap()
    wkT = nc.dram_tensor("wkT", [P, CO, C], BF16, kind="ExternalInput").ap()
    wvT = nc.dram_tensor("wvT", [P, CO, C], BF16, kind="ExternalInput").ap()
    bq = nc.dram_tensor("bq", [P, CO], F32, kind="ExternalInput").ap()
    bk = nc.dram_tensor("bk", [P, CO], F32, kind="ExternalInput").ap()
    bvb = nc.dram_tensor("bvb", [P, C], F32, kind="ExternalInput").ap()
    ones = nc.dram_tensor("ones", [1, P], F32R, kind="ExternalInput").ap()
    # output likewise packed [ci, qc, ct, n'], in bf16 (halves the final
    # store; quantization adds ~1e-3 norm err vs a 2e-2 gate); host unpacks
    out = nc.dram_tensor("out", [P, NQC, CO, QC], BF16,
                         kind="ExternalOutput").ap()

    with tile.TileContext(nc) as tc:
        with (
            nc.allow_low_precision(reason="fp32r/bf16 matmul operands"),
            tc.tile_pool(name="consts", bufs=1) as consts,
            tc.tile_pool(name="persist", bufs=1) as persist,
        ):
            # ---- constants (issue order matters: the first projection only
            # needs wqT + bq + the first qf chunk, so those go first and the
            # remaining weights ride behind the qf/kf streams) ----
            wqT_sb = consts.tile([P, CO, C], BF16, name="wqT_sb")
            nc.gpsimd.dma_start(wqT_sb[:], wqT[:])
            bq_sb = consts.tile([P, CO], F32, name="bq_sb")
            wkT_sb = consts.tile([P, CO, C], BF16, name="wkT_sb")
            wvT_sb = consts.tile([P, CO, C], BF16, name="wvT_sb")
            bk_sb = consts.tile([P, CO], F32, name="bk_sb")
            bvb_sb = consts.tile([P, C], F32, name="bvb_sb")
            ones_sb = consts.tile([1, P], F32R, name="ones_sb")
            # ---- PE warm-up, issued FIRST so PE starts the moment its
            # sequencer comes up. A burst of >=~26 junk matmuls at kernel
            # start locks the tensor engine's fast clock state (steady
            # 221ns/512-row matmul vs a permanent 263ns plateau without it:
            # +29us whole-kernel). 12 matmuls are NOT enough to trigger it.
            # Results are never read; the PSUM bank is reclaimed on pool
            # close. memset only supports f32 set-values, so the f32r tile
            # is filled via a converting copy.
            junk_f = consts.tile([P, QC], F32, name="junk_f")
            nc.vector.memset(junk_f[:], 1.0)
            junk_sb = consts.tile([P, QC], F32R, name="junk_sb")
            nc.vector.tensor_copy(out=junk_sb[:], in_=junk_f[:])
            with tc.tile_pool(name="warm_ps", bufs=1, space="PSUM") as warm_pool:
                warm_ps = warm_pool.tile([1, QC], F32, name="warm_ps")
                for i in range(N_WARM):
                    nc.tensor.matmul(
                        warm_ps[:], junk_sb[:, :1], junk_sb[:],
                        start=True, stop=True, skip_group_check=True,
                    )

            b0_sb = consts.tile([P, 1], F32, name="b0_sb")
            nc.vector.memset(b0_sb[:], EXP_SHIFT)
            onec_sb = consts.tile([P, 1], BF16, name="onec_sb")
            nc.vector.tensor_copy(out=onec_sb[:], in_=junk_f[:, :1])
            # dummy activation: pulls the ~1.3us LoadActFuncSet into the
            # initial DMA-wait window instead of blocking the first relu
            warm_sb = consts.tile([P, 1], F32, name="warm_sb")
            nc.scalar.activation(warm_sb[:], b0_sb[:], AF.Relu)

            # ---- persistent activations ----
            q_sb = persist.tile([P, CO, NQ], BF16, name="q_sb")
            k_sb = persist.tile([P, CO, NK], BF16, name="k_sb")
            vT_sb = persist.tile([P, KT, C], BF16, name="vT_sb")

            # ---- projections (staging pool scoped so its SBUF is reused) ----
            with (
                tc.tile_pool(name="staging", bufs=1) as staging,
                tc.tile_pool(name="proj_ps", bufs=1, space="PSUM") as proj_ps,
            ):
                # Input DMA plan. Each dma_start costs ~650ns of serial SP
                # dispatch, so: few DMAs, the first-needed bytes first
                # (wqT+bq+qf chunk 1), and strictly need-before order.
                qf_sb = staging.tile([P, CO, NQ], BF16, name="qf_sb")
                kf_sb = staging.tile([P, CO, NK], BF16, name="kf_sb")
                nqp, nkp = NQ // QP, NK // KP_
                nc.gpsimd.dma_start(bq_sb[:], bq[:])
                for p in range(QP):
                    nc.sync.dma_start(
                        qf_sb[:, :, p * nqp:(p + 1) * nqp], qf[:, p])
                nc.sync.dma_start(kf_sb[:, :, :nkp], kf[:, 0])
                nc.gpsimd.dma_start(wkT_sb[:], wkT[:])
                # wvT ahead of the tiny bias transfers: its late arrival was
                # the suspected cause of a recurring stall before the first
                # v-proj matmul (each dma_start is ~650ns of serial dispatch)
                nc.gpsimd.dma_start(wvT_sb[:], wvT[:])
                nc.gpsimd.dma_start(bk_sb[:], bk[:])
                nc.gpsimd.dma_start(bvb_sb[:], bvb[:])
                nc.gpsimd.dma_start(ones_sb[:], ones[:])
                for p in range(1, KP_):
                    nc.sync.dma_start(
                        kf_sb[:, :, p * nkp:(p + 1) * nkp], kf[:, p])

                def proj_iter(j, w_sb, bias_sb, dst, src_sb, oo1_act=False):
                    # one [*, QC] chunk of a q/k projection; relu+bias for
                    # oo=0 runs on ACT, oo=1 on DVE (or ACT for a couple of
                    # chunks, to even out the two engines' proj-phase load)
                    for oo in range(CO):
                        ps = proj_ps.tile([P, QC], F32, tag="pj", bufs=2,
                                          name=f"ps_{j}_{oo}")
                        for co in range(CO):
                            nc.tensor.matmul(
                                ps[:],
                                w_sb[:, co, oo * P:(oo + 1) * P],
                                src_sb[:, co, j * QC:(j + 1) * QC],
                                start=(co == 0), stop=(co == CO - 1),
                            )
                        if oo == 0 or oo1_act:
                            nc.scalar.activation(
                                dst[:, oo, j * QC:(j + 1) * QC], ps[:], AF.Relu,
                                bias=bias_sb[:, oo:oo + 1],
                            )
                        else:
                            nc.vector.tensor_scalar(
                                dst[:, oo, j * QC:(j + 1) * QC], ps[:],
                                bias_sb[:, oo:oo + 1], 0.0,
                                mybir.AluOpType.add, mybir.AluOpType.max,
                            )

                def vt_pair(kp):
                    # vT = relu(kf.T @ Wv'.T + bias_v): [n, o], n on partitions.
                    # bias_v varies along the free dim here, so it is added
                    # from a host-broadcast tile on DVE, then relu on ACT
                    # (writing bf16) -- no PE bias matmul needed.
                    psv = proj_ps.tile([P, 2, C], F32, tag="pv", bufs=3,
                                       name=f"psv_{kp}")
                    for half in range(2):
                        kt = 2 * kp + half
                        for co in range(CO):
                            nc.tensor.matmul(
                                psv[:, half, :],
                                kf_sb[:, co, kt * P:(kt + 1) * P],
                                wvT_sb[:, co, :],
                                start=(co == 0), stop=(co == CO - 1),
                            )
                    vtmp = staging.tile([P, 2, C], F32, tag="vtmp", bufs=3,
                                        name=f"vtmp_{kp}")
                    nc.vector.tensor_tensor(
                        vtmp[:], psv[:],
                        bvb_sb[:, None, :].to_broadcast((P, 2, C)),
                        ADD,
                    )
                    nc.scalar.activation(
                        vT_sb[:, 2 * kp:2 * kp + 2, :], vtmp[:], AF.Relu)

                # q = relu(Wq' @ qf + bq): [o, n] with o on partitions.
                # j-major so each arriving qf/kf chunk is fully consumed at
                # once; the vT pairs for chunk j of kf ride along with proj-k
                # so PE work fills the relu latency.
                for j in range(NQ // QC):
                    proj_iter(j, wqT_sb, bq_sb, q_sb, qf_sb)
                for j in range(NK // QC):
                    proj_iter(j, wkT_sb, bk_sb, k_sb, kf_sb,
                              oo1_act=j in (3, 6))
                    vt_pair(2 * j)
                    vt_pair(2 * j + 1)

            # ---- attention ----
            with (
                tc.tile_pool(name="expp", bufs=1) as expp,
                tc.tile_pool(name="outp", bufs=1) as outp,
                tc.tile_pool(name="attn_ps", bufs=1, space="PSUM") as attn_ps,
            ):
                # Software pipeline: step s emits sim+exp for chunk s
                # interleaved (at k-pair granularity) with the ctx matmuls
                # consuming chunk s-1's exp tiles. PE's ctx work fills the
                # ACT-exp latency that otherwise stalls the sim phase, and
                # the per-chunk phase boundaries disappear.
                exp_pairs = {}     # qc -> list of pair tiles
                sum_tiles = {}     # qc -> [P, QC] bf16 total of all 32 exps
                carry = {}         # qc -> binary-counter merge tree state

                def emit_sim_pair(qc, kp):
                    qs = slice(qc * QC, (qc + 1) * QC)
                    ps = attn_ps.tile([P, 2, QC], F32, tag="sim", bufs=2,
                                      name=f"pss_{qc}_{kp}")
                    for half in range(2):
                        kt = 2 * kp + half
                        for co in range(CO):
                            nc.tensor.matmul(
                                ps[:, half, :],
                                k_sb[:, co, kt * P:(kt + 1) * P],
                                q_sb[:, co, qs],
                                start=(co == 0), stop=(co == CO - 1),
                            )
                    et = expp.tile([P, 2, QC], BF16, tag="expT", bufs=24,
                                   name=f"expT_{qc}_{kp}")
                    nc.scalar.activation(et[:], ps[:], AF.Exp,
                                         bias=b0_sb[:], scale=float(SCALE))
                    exp_pairs.setdefault(qc, []).append(et)
                    # row-sum accumulation on DVE (bf16, 4x element mode):
                    # halves-add, then binary-counter merge. After the last
                    # pair, carry[4] holds the chunk's full exp sum.
                    h = expp.tile([P, QC], BF16, tag="hsum", bufs=3,
                                  name=f"h_{qc}_{kp}")
                    nc.vector.tensor_tensor(h[:], et[:, 0, :], et[:, 1, :], ADD)
                    node, lvl, c = h, 0, carry[qc]
                    while c[lvl] is not None:
                        prev, c[lvl] = c[lvl], None
                        nxt = expp.tile([P, QC], BF16, tag=f"lv{lvl + 1}",
                                        bufs=2, name=f"s_{qc}_{kp}_{lvl + 1}")
                        nc.vector.tensor_tensor(nxt[:], prev[:], node[:], ADD)
                        node, lvl = nxt, lvl + 1
                    c[lvl] = node
                    if kp == NP - 1:
                        sum_tiles[qc] = c[4]

                def emit_ctx_pair(qc, kp, ctx_ps):
                    for half in range(2):
                        kt = 2 * kp + half
                        e = exp_pairs[qc][kp][:, half, :]
                        for ct in range(CO):
                            nc.tensor.matmul(
                                ctx_ps[ct][:],
                                vT_sb[:, kt, ct * P:(ct + 1) * P],
                                e,
                                start=(kt == 0), stop=(kt == KT - 1),
                                skip_group_check=True,
                            )

                def emit_norm_sums(qcp):
                    # sums -> reciprocal, early: S was finished by DVE at the
                    # end of the previous step, so the slow reciprocal
                    # overlaps the ctx matmuls instead of stalling the tail.
                    S = sum_tiles.pop(qcp)
                    sums_ps = attn_ps.tile([1, QC], F32, tag="sums", bufs=2,
                                           name=f"psS_{qcp}")
                    nc.tensor.matmul(sums_ps[:], onec_sb[:], S[:],
                                     start=True, stop=True)
                    recip = outp.tile([1, QC], F32R, tag="recip",
                                      bufs=2, name=f"recip_{qcp}")
                    nc.vector.reciprocal(recip[:], sums_ps[:])
                    return recip

                def emit_norm_bcast(qcp, recip):
                    # broadcast 1/sums to all partitions (K=1 matmul) once
                    # the reciprocal has finished
                    bc_ps = attn_ps.tile([P, QC], F32, tag="sim",
                                         bufs=2, name=f"psb_{qcp}")
                    nc.tensor.matmul(bc_ps[:], ones_sb[:], recip[:],
                                     start=True, stop=True)
                    bc_sb = outp.tile([P, QC], F32, tag="bc",
                                      bufs=2, name=f"bc_{qcp}")
                    nc.vector.tensor_copy(out=bc_sb[:], in_=bc_ps[:])
                    return bc_sb

                def emit_last_chunk(qcp, ctx_ps):
                    # Final chunk, ct-major: ct0's full 32-tile accumulation,
                    # normalize and store drain underneath ct1's matmuls, so
                    # the end-of-kernel chain is only ct1's mul + DMA.
                    recip = emit_norm_sums(qcp)
                    ot = outp.tile([P, CO, QC], BF16, tag="out", bufs=2,
                                   name=f"out_{qcp}")
                    bc_sb = None
                    for ct in range(CO):
                        for kp in range(NP):
                            for half in range(2):
                                kt = 2 * kp + half
                                e = exp_pairs[qcp][kp][:, half, :]
                                nc.tensor.matmul(
                                    ctx_ps[ct][:],
                                    vT_sb[:, kt, ct * P:(ct + 1) * P],
                                    e,
                                    start=(kt == 0), stop=(kt == KT - 1),
                                    skip_group_check=True,
                                )
                        if ct == 0:
                            # broadcast between the ct passes: PE arrives
                            # ~7us into the step, well after the reciprocal
                            # retires, so no stall; ct0's mul+store still
                            # drain under ct1's matmuls
                            bc_sb = emit_norm_bcast(qcp, recip)
                            nc.vector.tensor_mul(ot[:, ct, :], ctx_ps[ct][:],
                                                 bc_sb[:])
                            nc.sync.dma_start(out[:, qcp, ct], ot[:, ct, :])
                        else:
                            # very last store, in column halves: the first
                            # half's DMA rides under the second half's mul
                            hq = QC // 2
                            for hh in range(2):
                                cs = slice(hh * hq, (hh + 1) * hq)
                                nc.vector.tensor_mul(
                                    ot[:, ct, cs], ctx_ps[ct][:, cs],
                                    bc_sb[:, cs])
                                nc.sync.dma_start(out[:, qcp, ct, cs],
                                                  ot[:, ct, cs])
                    exp_pairs.pop(qcp)

                ctx_live = None  # (qc, ctx_ps, recip, [bc_sb]) being accumulated
                for s in range(NQC + 1):
                    if s > 0:
                        qcp = s - 1
                        ctx_ps = [
                            attn_ps.tile([P, QC], F32, tag="ctx", bufs=2,
                                         name=f"psc_{qcp}_{ct}")
                            for ct in range(CO)
                        ]
                        ctx_live = (qcp, ctx_ps, None, [None])
                    if s < NQC:
                        carry[s] = [None] * 5
                    if s == NQC:
                        emit_last_chunk(qcp, ctx_ps)
                        ctx_live = None
                        break
                    for kp in range(NP):
                        if s < NQC:
                            emit_sim_pair(s, kp)
                        if ctx_live is not None:
                            qcp, ctx_ps, recip, bc_box = ctx_live
                            emit_ctx_pair(qcp, kp, ctx_ps)
                            if kp == 1:
                                recip = emit_norm_sums(qcp)
                                ctx_live = (qcp, ctx_ps, recip, bc_box)
                            elif kp == BC_KP:
                                bc_box[0] = emit_norm_bcast(qcp, recip)
                    if ctx_live is not None:
                        qcp, ctx_ps, recip, bc_box = ctx_live
                        ot = outp.tile([P, CO, QC], BF16, tag="out", bufs=2,
                                       name=f"out_{qcp}")
                        # per-ct DMA so the ct=0 store overlaps the ct=1 mul
                        # (shortens the end-of-kernel chain)
                        for ct in range(CO):
                            nc.vector.tensor_mul(ot[:, ct, :], ctx_ps[ct][:],
                                                 bc_box[0][:])
                            nc.sync.dma_start(out[:, qcp, ct], ot[:, ct, :])
                        exp_pairs.pop(qcp)
                        ctx_live = None

    nc.compile()
    return nc


_PROGRAM = None


def _get_program():
    global _PROGRAM
    if _PROGRAM is None:
        _PROGRAM = _build_program()
    return _PROGRAM


def _prepare_in_maps(
    query_feats, key_feats, Wq, Wk, Wv,
    scale_q, bias_q, scale_k, bias_k, scale_v, bias_v,
):
    import ml_dtypes

    bf16 = ml_dtypes.bfloat16
    f32 = np.float32
    qf_all = np.asarray(query_feats, f32).reshape(B, C, NK)
    kf_all = np.asarray(key_feats, f32).reshape(B, C, NK)

    def pack_feat(x, pieces):
        # [C, N] -> [ci, piece, co, n'] so each DMA piece is 128 long rows
        n = x.shape[1]
        return np.ascontiguousarray(
            x.reshape(CO, P, pieces, n // pieces).transpose(1, 2, 0, 3)
        ).astype(bf16)

    def pack_w(wT):
        # [C, C] -> [ci, co, o]
        return np.ascontiguousarray(
            wT.reshape(CO, P, C).transpose(1, 0, 2)).astype(bf16)

    wqT = pack_w((np.asarray(scale_q, f32)[:, None] * np.asarray(Wq, f32)).T)
    wkT = pack_w((np.asarray(scale_k, f32)[:, None] * np.asarray(Wk, f32)).T)
    wvT = pack_w((np.asarray(scale_v, f32)[:, None] * np.asarray(Wv, f32)).T)
    bq2 = np.ascontiguousarray(np.asarray(bias_q, f32).reshape(CO, P).T)
    bk2 = np.ascontiguousarray(np.asarray(bias_k, f32).reshape(CO, P).T)
    bvb = np.broadcast_to(np.asarray(bias_v, f32)[None, :], (P, C)).copy()
    ones = np.ones((1, P), f32)

    shared = dict(wqT=wqT, wkT=wkT, wvT=wvT, bq=bq2, bk=bk2,
                  bvb=bvb, ones=ones)
    in_maps = []
    for core in range(8):
        b, h = divmod(core, 2)
        in_maps.append(dict(
            qf=pack_feat(qf_all[b][:, h * NQ:(h + 1) * NQ], QP),
            kf=pack_feat(kf_all[b], KP_),
            **shared,
        ))
    return in_maps


def run(inputs: dict, trace: bool = False):
    """Compile (cached) + run on 8 cores. Returns (output, BassKernelResults)."""
    nc = _get_program()
    in_maps = _prepare_in_maps(**inputs)
    res = run_bass_kernel_spmd(nc, in_maps, core_ids=list(range(8)), trace=trace)
    full = np.empty((B, C, NK), np.float32)
    for core in range(8):
        b, h = divmod(core, 2)
        # device wrote bf16 [ci, qc, ct, n']; un-permute to f32 [C, NQ]
        o = np.asarray(res.results[core]["out"],
                       np.float32).reshape(P, NQC, CO, QC)
        full[b][:, h * NQ:(h + 1) * NQ] = (
            o.transpose(2, 0, 1, 3).reshape(C, NQ))
    return full.reshape(B, C, H, W), res


def kernel(**inputs) -> np.ndarray:
    return run(inputs)[0]
